# revision 17
# baseline (speedup 1.0000x reference)
"""AWQ int4 linear + fused LoRA on 8 Trainium2 NeuronCores.

Computes out = x @ dequant(qweight, qzeros, scales) + (x @ lora_a) @ lora_b
with tensor-parallel column sharding over N (no collectives needed).

Per-core device kernel:
  Phase A: dequantize the per-core weight shard W = q*s + (A@B - z*s) into
    SBUF (bf16).  The LoRA product and the zero-point correction ride ONE
    matmul per tile: lhsT = [A^T ; IND ; IND] (128 x K) against
    rhs = [B ; c_hi ; c_lo] (128 x NP), where IND is the group-indicator
    and c = -z*s is split hi/lo to keep ~16 mantissa bits.  A second
    matmul broadcasts s across partitions; two DVE ops merge:
    W = q * s_bcast + (AB + c).
  Phase B: dense bf16 GEMM x^T-tiles (k-major, pre-cast to bf16 on host,
    HWDGE loads) against the resident W, accumulating fp32 in PSUM
    (full-width 1376-col psum tile, single evict per token tile).
"""

import sys

if "/opt/trn_rl_repo" not in sys.path:
    sys.path.insert(0, "/opt/trn_rl_repo")

import numpy as np

P = 128
N_CORES = 8
T_FULL, K_FULL, N_FULL = 8192, 4096, 11008
R_FULL = 64
NSH = N_FULL // N_CORES  # 1376 columns per core
NP_FULL = NSH


def _n_slices(NP, max_free=512):
    out = []
    off = 0
    while off < NP:
        ns = min(max_free, NP - off)
        out.append((off, ns))
        off += ns
    return out


def _patched_tile_context(tile_mod, nc):
    """TileContext whose tail drain keeps <=1 sem wait per SP instruction
    (this walrus build rejects >2 sync waits on a Drain)."""
    from bass_rust import ScopedClock, SyncInfo

    class TileContextPatched(tile_mod.TileContext):
        def _drain_and_barrier(self, tick_clock, wait_clock):
            drain_inst = self.nc.sync.drain()
            wait_clock.add_sem_waits(
                drain_inst.ins, ScopedClock({None: tick_clock.global_clock})
            )
            si = drain_inst.ins.sync_info
            waits = list(si.on_wait) if si is not None else []
            if len(waits) > 1:
                drain_inst.ins.sync_info.on_wait = waits[:1]
                for w in waits[1:]:
                    nop = self.nc.sync.nop()
                    nop.ins.sync_info = SyncInfo(on_wait=[w], on_update=[])

            self.nc.all_engine_barrier()
            assert self.sems is not None
            popped = self.nc._tile_sem_poison_stack.pop()
            assert popped is self._sem_poison
            self.nc.clear_and_free_semaphores(list(self.sems.allocated().values()))
            self.nc.all_engine_barrier()

    return TileContextPatched(nc)


def _split_multi_waits(nc, max_waits=1):
    """This walrus build rejects instructions carrying more than ~1-2 sem
    waits ('Too many sync wait commands').  Move extra waits onto standalone
    EventSemaphore instructions inserted just before, on the same engine —
    engines execute their stream in order, so this is semantically identical.
    """
    from concourse import mybir

    n_split = 0
    for f in nc.m.functions:
        for bb in f.blocks:
            insts = list(bb.instructions)
            out, changed = [], False
            for inst in insts:
                si = inst.sync_info
                if si is not None and len(si.on_wait) > max_waits:
                    waits = list(si.on_wait)
                    for w in waits[:-max_waits]:
                        n_split += 1
                        nop = mybir.InstEventSemaphore(
                            name=f"{inst.name}-ws{n_split}", ins=[], outs=[])
                        nop.engine = inst.engine
                        nop.sync_info = mybir.SyncInfo(on_wait=[w], on_update=[])
                        out.append(nop)
                    si.on_wait = waits[-max_waits:]
                    changed = True
                out.append(inst)
            if changed:
                bb.instructions = out
    return n_split


def _dedupe_ldweights(nc):
    """Legalization pairs every InstMatmult with its own InstLdweights, even
    when consecutive matmuls share the same stationary operand (our phase B
    issues 3 N-slice matmuls per x-tile).  Weights persist in the PE array
    across matmuls, so a reload of the identical physical AP is pure
    overhead (~P/1.2GHz each on the PE queue).  Drop such repeats, keeping
    their sem waits/updates on a nop so the sync graph is unchanged."""
    from concourse import mybir

    n = 0
    for f in nc.m.functions:
        for bb in f.blocks:
            insts = list(bb.instructions)
            out, changed = [], False
            prev_key = None
            for inst in insts:
                if isinstance(inst, mybir.InstLdweights):
                    key = (str(inst.ins[0]), str(inst.perf_mode),
                           str(inst.is_transpose), str(inst.tile_position),
                           str(inst.tile_size))
                    if key == prev_key:
                        si = inst.sync_info
                        if si is not None and (si.on_wait or si.on_update):
                            nop = mybir.InstEventSemaphore(
                                name=f"{inst.name}-lw", ins=[], outs=[])
                            nop.engine = inst.engine
                            nop.sync_info = si
                            out.append(nop)
                        n += 1
                        changed = True
                        continue
                    prev_key = key
                elif str(getattr(inst, "engine", None)) == "EngineType.PE":
                    if isinstance(inst, mybir.InstMatmult):
                        if getattr(inst, "is_transpose", False):
                            prev_key = None  # transpose uses the weight path
                    elif not isinstance(inst, mybir.InstEventSemaphore):
                        prev_key = None  # drains/branches: be conservative
                out.append(inst)
            if changed:
                bb.instructions = out
    return n


def _batch_mm_incs(nc):
    """Defer per-matmul tick-semaphore increments to segment boundaries.

    Every InstMatmult carries `then_inc(PE_nn, 1)`; waiters use sem-ge-imm
    thresholds on that counter.  For runs of consecutive PE {Ldweights,
    Matmult} instructions we strip the per-MM incs and emit one batched inc
    on the last MM of each segment, splitting segments EXACTLY at every
    counter value some instruction waits on — so each waiter still fires at
    the completion of the same matmul as before (intermediate counter values
    are unobservable).  This removes ~6k serialized EVT_SEM writes from the
    PE queue per iteration."""
    from concourse import mybir

    n_batched = 0
    for f in nc.m.functions:
        for bb in f.blocks:
            insts = list(bb.instructions)
            # 1. find the PE tick semaphore name (the one PE matmuls inc)
            sem_name = None
            for inst in insts:
                if isinstance(inst, mybir.InstMatmult):
                    si = inst.sync_info
                    if si is not None and len(si.on_update) == 1:
                        sem_name = si.on_update[0].ant_name
                        break
            if sem_name is None:
                continue
            # 2. all observed wait thresholds on that semaphore
            thresholds = set()
            for inst in insts:
                si = inst.sync_info
                if si is None:
                    continue
                for w in si.on_wait:
                    if w.ant_name == sem_name and w.wait_value is not None:
                        thresholds.add(int(w.wait_value))
            # 3. walk PE instructions, batching runs of LDW/MM
            count = 0  # running counter value of sem_name

            def mm_inc(inst):
                si = inst.sync_info
                if (isinstance(inst, mybir.InstMatmult) and si is not None
                        and len(si.on_update) == 1
                        and si.on_update[0].ant_name == sem_name
                        and si.on_update[0].update_mode == "sem-inc"):
                    return int(si.on_update[0].update_value)
                return 0

            run = []  # [(inst, inc)] of current LDW/MM run

            def flush_run():
                nonlocal n_batched, count
                mms = [(i, v) for (i, v) in run if v > 0]
                if len(mms) >= 2:
                    # split at thresholds (and a size cap); batch segments
                    seg = []
                    c = count
                    for idx, (i, v) in enumerate(mms):
                        c += v
                        seg.append((i, v))
                        if (c in thresholds or idx == len(mms) - 1
                                or len(seg) >= 48):
                            if len(seg) > 1:
                                total = sum(vv for (_, vv) in seg)
                                for (si_inst, _) in seg[:-1]:
                                    si_inst.sync_info.on_update = []
                                last = seg[-1][0]
                                upd = last.sync_info.on_update[0]
                                upd.update_value = total
                                last.sync_info.on_update = [upd]
                                n_batched += len(seg) - 1
                            seg = []
                    count = c
                else:
                    count += sum(v for (_, v) in run)
                run.clear()

            for inst in insts:
                if str(getattr(inst, "engine", None)) != "EngineType.PE":
                    continue
                si = inst.sync_info
                is_ldw_mm = isinstance(
                    inst, (mybir.InstMatmult, mybir.InstLdweights))
                clean = (is_ldw_mm
                         and (si is None or not si.on_wait)
                         and (isinstance(inst, mybir.InstLdweights)
                              or mm_inc(inst) == 1))
                if clean:
                    run.append((inst, mm_inc(inst)))
                else:
                    flush_run()
                    # non-clean instruction may still inc the counter
                    if si is not None:
                        for u in si.on_update:
                            if (u.ant_name == sem_name
                                    and u.update_mode == "sem-inc"):
                                count += int(u.update_value)
            flush_run()
    return n_batched


def _strip_mm_incs(nc):
    """Remove per-matmul tick-sem increments nobody observes.

    Tile gives every InstMatmult `then_inc(PE_nn, 1)`; waiters use
    sem-ge-imm thresholds.  Keep only the incs whose cumulative count is
    referenced by some wait (plus the last inc per block), then renumber
    every wait to the kept-inc index whose instruction crosses its original
    threshold — each waiter fires at the completion of the exact same
    matmul as before.  For_i resets sems per iteration (reset block w/
    all-engine barrier), so per-block counting is the right scope."""
    from concourse import mybir

    # find the PE tick sem name
    sem_name = None
    for f in nc.m.functions:
        for bb in f.blocks:
            for inst in bb.instructions:
                if isinstance(inst, mybir.InstMatmult):
                    si = inst.sync_info
                    if si is not None and len(si.on_update) == 1:
                        sem_name = si.on_update[0].ant_name
                        break
            if sem_name:
                break
        if sem_name:
            break
    if sem_name is None:
        return 0

    # global thresholds on that sem (waits can live in other blocks, e.g.
    # the For_i reset block)
    all_waits = []  # (wait_obj)
    for f in nc.m.functions:
        for bb in f.blocks:
            for inst in bb.instructions:
                si = inst.sync_info
                if si is None:
                    continue
                for w in si.on_wait:
                    if w.ant_name == sem_name and w.wait_value is not None:
                        all_waits.append(w)
    thresholds = sorted({int(w.wait_value) for w in all_waits})

    # walk the block containing the incs (assume single body block holds
    # them; handle every block independently but build one mapping — incs
    # only exist in the body block in this kernel)
    n_removed = 0
    mapping = {}  # old cumulative count -> new cumulative count
    for f in nc.m.functions:
        for bb in f.blocks:
            inc_insts = []
            for inst in bb.instructions:
                si = inst.sync_info
                if si is None:
                    continue
                for u in si.on_update:
                    if (u.ant_name == sem_name
                            and u.update_mode == "sem-inc"):
                        assert int(u.update_value) == 1
                        inc_insts.append(inst)
            if not inc_insts:
                continue
            assert not mapping, "incs in more than one block"
            thr = set(thresholds)
            keep = []
            for i, inst in enumerate(inc_insts):
                c = i + 1
                is_kept = (c in thr or i == len(inc_insts) - 1
                           or not isinstance(inst, mybir.InstMatmult))
                if is_kept:
                    keep.append(c)
                else:
                    si = inst.sync_info
                    si.on_update = [u for u in si.on_update
                                    if u.ant_name != sem_name]
                    n_removed += 1
            # old count w is crossed by the first kept inc with c >= w;
            # its new cumulative value is its index+1 in `keep`
            import bisect
            for w in thresholds:
                pos = bisect.bisect_left(keep, w)
                assert pos < len(keep), (w, keep[-1:])
                mapping[w] = pos + 1
    for w in all_waits:
        w.wait_value = mapping[int(w.wait_value)]
    return n_removed


ALL_FEATURES = frozenset({"phase_a", "xload", "mm", "evict", "store"})


def build_bass(T=T_FULL, K=K_FULL, NP=NP_FULL, R=R_FULL, TSUP=256,
               num_devices=N_CORES, split_waits=True, repeat=1,
               loop_repeat=1, features=ALL_FEATURES, xb_bufs=3,
               mm_order="slice_tiles", batch_incs=False, acc_mode="accum",
               strip_incs=False, evict_engine="dve"):
    # mm_order="slice_tiles" (j-inner, per-slice PSUM tiles) keeps one PSUM
    # bank as the matmul target for 32 consecutive MMs; the j-outer order
    # cycled banks every MM and measured ~70ns/MM slower (1604us vs 1193us
    # for the bare MM loop).  NOTE: a "slice_outer" variant that sliced one
    # 3-bank [P, NP] psum tile per slice-group produced NaN output on HW
    # (walrus PSUM-group legalization at memref granularity, presumably) —
    # per-slice one-bank tiles are required.
    # batch_incs=True is REJECTED by walrus ("UpdateValue == 1" assert) —
    # kept only for reference, do not enable.
    """Build the per-core Bass program (SPMD: all cores run this)."""
    import concourse.bass as bass
    import concourse.tile as tile
    from concourse import mybir

    NG = K // P  # k-tiles; == quant groups (group size 128)
    assert T % TSUP == 0 and TSUP % P == 0
    f32, bf16, fp16 = mybir.dt.float32, mybir.dt.bfloat16, mybir.dt.float16

    nc = bass.Bass("TRN2", target_bir_lowering=False, debug=False,
                   num_devices=num_devices)
    xt_d = nc.dram_tensor("xt", [K, T], bf16, kind="ExternalInput")
    q_d = nc.dram_tensor("q", [K, NP], bf16, kind="ExternalInput")
    sx_d = nc.dram_tensor("sx", [2 * (K // P), NP], bf16, kind="ExternalInput")
    atx_d = nc.dram_tensor("atx", [P, K], bf16, kind="ExternalInput")
    bcx_d = nc.dram_tensor("bcx", [P, NP], bf16, kind="ExternalInput")
    out_d = nc.dram_tensor("out", [T, NP], f32, kind="ExternalOutput")

    slices = _n_slices(NP)

    from contextlib import ExitStack

    tc = _patched_tile_context(tile, nc)
    with tc, ExitStack() as ctx:
        const = ctx.enter_context(tc.tile_pool(name="const", bufs=1))
        # lhsT for the LoRA + zero-point correction: [A^T ; IND ; IND]
        atx_sb = const.tile([P, K], bf16, name="atx_sb")
        nc.sync.dma_start(atx_sb[:], atx_d.ap())
        # rhs: [B ; c_hi ; c_lo], host-precomputed
        bcx_sb = const.tile([P, NP], bf16, name="bcx_sb")
        nc.sync.dma_start(bcx_sb[:], bcx_d.ap())
        # scales hi/lo split [s_hi ; s_lo] (2NG x NP) bf16 — the broadcast
        # matmul against [IND ; IND] reconstructs s_hi+s_lo in fp32 PSUM
        # (~16 mantissa bits) at bf16 matmul speed (no f32r self-loading)
        sx_sb = const.tile([2 * NG, NP], bf16, name="sx_sb")
        nc.sync.dma_start(sx_sb[:], sx_d.ap())
        # [IND ; IND] lhsT for the s broadcast: rows R..R+2NG of atx
        indx_sb = const.tile([2 * NG, K], bf16, name="indx_sb")
        nc.sync.dma_start(indx_sb[:], atx_d.ap()[R:R + 2 * NG, :])

        wpool = ctx.enter_context(tc.tile_pool(name="wpool", bufs=1))
        W_sb = wpool.tile([P, NG, NP], bf16, name="W_sb")
        if "phase_a" not in features:
            nc.vector.memset(W_sb[:, 0:1, 0:1], 0.0)

        # All working pools coexist at one scope: phase A and phase B tiles
        # never alias addresses, so the scheduler can overlap the phases.
        deq = ctx.enter_context(tc.tile_pool(name="deq", bufs=2))
        ps_pool = ctx.enter_context(tc.tile_pool(name="ps", bufs=1,
                                                 space="PSUM"))
        xb = ctx.enter_context(tc.tile_pool(name="xb", bufs=xb_bufs))
        ob = ctx.enter_context(tc.tile_pool(name="ob", bufs=2))

        from contextlib import nullcontext

        for rep in range(repeat):
          # loop_repeat>1 re-runs the body via a hardware loop (constant
          # program size; used by the timing harness for slope measurement)
          with (tc.For_i(0, loop_repeat, 1) if loop_repeat > 1
                else nullcontext()):
            # ---- Phase A: dequant + LoRA fold ----
            if "phase_a" in features:
                for j in range(NG):
                    q_t = deq.tile([P, NP], bf16, name="q_t")
                    # NOTE: routing this load off the SP HWDGE ring was tried
                    # both ways and rejected: via nc.scalar it costs ACT
                    # sequencer time that delays the sb_t evicts (+28us in
                    # the cost model); via nc.gpsimd (SWDGE) walrus fails to
                    # compile the program.  SP ring it is.
                    nc.sync.dma_start(q_t[:], q_d.ap()[j * P:(j + 1) * P, :])
                    wj = W_sb[:, j]
                    for (off, ns) in slices:
                        # broadcast s_j across partitions: psum[p,n] = s[j,n]
                        ps_s = ps_pool.tile([P, 512], f32, name="ps_s",
                                            bufs=1)
                        nc.tensor.matmul(
                            ps_s[:, :ns],
                            lhsT=indx_sb[:, j * P:(j + 1) * P],
                            rhs=sx_sb[:, off:off + ns],
                            start=True, stop=True,
                        )
                        # LoRA chunk + zero-point: [A^T;IND;IND]^T @ [B;c]
                        ps_ab = ps_pool.tile([P, 512], f32, name="ps_ab",
                                             bufs=1)
                        nc.tensor.matmul(
                            ps_ab[:, :ns],
                            lhsT=atx_sb[:, j * P:(j + 1) * P],
                            rhs=bcx_sb[:, off:off + ns],
                            start=True, stop=True,
                        )
                        # evict both psums to 16-bit SBUF on the (idle) ACT
                        # engine so BOTH DVE ops run in 2x 16-bit mode.
                        # ab goes to fp16 (10 mantissa bits) to keep most of
                        # the c_hi/c_lo zero-point precision; s to bf16.
                        sb_t = deq.tile([P, 512], bf16, name="sb_t")
                        nc.scalar.copy(sb_t[:, :ns], ps_s[:, :ns])
                        ab_t = deq.tile([P, 512], fp16, name="ab_t")
                        nc.scalar.copy(ab_t[:, :ns], ps_ab[:, :ns])
                        qs_t = deq.tile([P, 512], bf16, name="qs_t")
                        nc.vector.tensor_mul(qs_t[:, :ns],
                                             q_t[:, off:off + ns],
                                             sb_t[:, :ns])
                        # W = qs + (A@B + c) chunk
                        nc.vector.tensor_add(wj[:, off:off + ns],
                                             qs_t[:, :ns],
                                             ab_t[:, :ns])

            # ---- Phase B: main GEMM ----
            if True:
                xt_r = xt_d.ap().rearrange("(j p) t -> p j t", p=P)
                for sidx in range(T // TSUP):
                    t0 = sidx * TSUP
                    x_t = xb.tile([P, NG, TSUP], bf16, name="x_t")
                    if "xload" in features:
                        nc.sync.dma_start(x_t[:], xt_r[:, :, t0:t0 + TSUP])
                    else:
                        nc.vector.memset(x_t[:, 0:1, 0:1], 0.0)
                    for tsub in range(TSUP // P):
                        out_t = ob.tile([P, NP], f32, name="out_t")
                        if mm_order == "slice_tiles":
                            # j-inner with a dedicated one-bank PSUM tile per
                            # slice: one bank stays the matmul target for 32
                            # consecutive MMs (the j-outer order cycled banks
                            # every MM: +70ns/MM, 1604us vs 1193us bare), and
                            # each slice is evicted as soon as its group ends.
                            for (off, ns) in slices:
                                pt = ps_pool.tile([P, 512], f32,
                                                  name="mm_ps", bufs=6)
                                if "mm" in features:
                                    for j in range(NG):
                                        # acc_mode="overwrite" is a TIMING-
                                        # ONLY ablation (start=True per MM
                                        # breaks the K-accumulation math)
                                        nc.tensor.matmul(
                                            pt[:, :ns],
                                            lhsT=x_t[:, j][:, tsub * P:
                                                           (tsub + 1) * P],
                                            rhs=W_sb[:, j][:, off:off + ns],
                                            start=(j == 0
                                                   or acc_mode == "overwrite"),
                                            stop=(j == NG - 1
                                                  or acc_mode == "overwrite"),
                                        )
                                if "evict" in features and "mm" in features:
                                    if evict_engine == "act":
                                        nc.scalar.copy(
                                            out_t[:, off:off + ns],
                                            pt[:, :ns])
                                    else:
                                        nc.vector.tensor_copy(
                                            out_t[:, off:off + ns],
                                            pt[:, :ns])
                        else:
                            pt = ps_pool.tile([P, NP], f32, name="mm_ps",
                                              bufs=2)
                            if "mm" in features:
                                for j in range(NG):
                                    lhsT = x_t[:, j][:, tsub * P:
                                                     (tsub + 1) * P]
                                    for (off, ns) in slices:
                                        nc.tensor.matmul(
                                            pt[:, off:off + ns],
                                            lhsT=lhsT,
                                            rhs=W_sb[:, j][:, off:off + ns],
                                            start=(j == 0),
                                            stop=(j == NG - 1),
                                        )
                            if "evict" in features and "mm" in features:
                                nc.vector.tensor_copy(out_t[:], pt[:])
                        if not ("evict" in features and "mm" in features):
                            nc.vector.memset(out_t[:, 0:1], 0.0)
                        if "store" in features:
                            nc.sync.dma_start(
                                out_d.ap()[t0 + tsub * P:t0 + (tsub + 1) * P, :],
                                out_t[:],
                            )
    # NOTE: an LDW-dedupe pass (drop repeated identical InstLdweights, keeping
    # one per 3-matmul slice group) was tried here and VERIFIED WRONG on HW:
    # rel err jumped to 7.4e-2 with no speedup — the per-matmul LDW pairing
    # is required by this toolchain/silicon and is already overlap-hidden.
    if split_waits:
        _split_multi_waits(nc)
    if batch_incs:
        _batch_mm_incs(nc)
    if strip_incs:
        _strip_mm_incs(nc)
    return nc


def _marshal_inputs(x, scales, lora_a, lora_b, qweight, qzeros,
                    n_cores=N_CORES, NP=NP_FULL):
    """Host-side sharding + layout prep (transpose / dtype cast / padding)."""
    import ml_dtypes

    bf16 = ml_dtypes.bfloat16
    x = np.asarray(x, dtype=np.float32)
    scales = np.asarray(scales, dtype=np.float32)
    lora_a = np.asarray(lora_a, dtype=np.float32)
    lora_b = np.asarray(lora_b, dtype=np.float32)
    qweight = np.asarray(qweight, dtype=np.int32)
    qzeros = np.asarray(qzeros, dtype=np.int32)

    K, N = qweight.shape
    NG = scales.shape[0]
    nsh = N // n_cores

    xt = np.ascontiguousarray(x.T).astype(bf16)     # [K, T] bf16
    indic = np.kron(np.eye(NG, dtype=np.float32),
                    np.ones((1, P), np.float32))    # [NG, NG*128]
    # lhsT [A^T ; IND ; IND]  (128 x K) bf16 — IND rows are exact in bf16
    atx = np.concatenate([lora_a.T, indic, indic], axis=0).astype(bf16)

    # zero-point correction c = -z*s, split hi+lo for ~16 mantissa bits
    c = -(qzeros.astype(np.float32) * scales)       # [NG, N]
    c_hi = c.astype(bf16)
    c_lo = (c - c_hi.astype(np.float32)).astype(bf16)

    # scales split hi+lo the same way (reconstructed as s_hi+s_lo in fp32
    # PSUM by the broadcast matmul, keeping ~16 mantissa bits)
    s_hi = scales.astype(bf16)
    s_lo = (scales - s_hi.astype(np.float32)).astype(bf16)

    qb = qweight.astype(bf16)                       # exact (values 0..15)

    in_maps = []
    for cidx in range(n_cores):
        lo, hi = cidx * nsh, (cidx + 1) * nsh
        q = np.ascontiguousarray(qb[:, lo:hi])
        sx = np.ascontiguousarray(
            np.concatenate([s_hi[:, lo:hi], s_lo[:, lo:hi]], axis=0))
        bcx = np.concatenate([lora_b[:, lo:hi].astype(bf16),
                              c_hi[:, lo:hi], c_lo[:, lo:hi]], axis=0)
        bcx = np.ascontiguousarray(bcx)             # [128, nsh] bf16
        in_maps.append({"xt": xt, "q": q, "sx": sx, "atx": atx, "bcx": bcx})
    return in_maps, nsh


_NC_CACHE = {}


def kernel(x, scales, lora_a, lora_b, qweight, qzeros):
    from concourse.bass_utils import run_bass_kernel_spmd

    in_maps, nsh = _marshal_inputs(x, scales, lora_a, lora_b, qweight, qzeros)
    key = "full"
    if key not in _NC_CACHE:
        _NC_CACHE[key] = build_bass()
    nc = _NC_CACHE[key]
    res = run_bass_kernel_spmd(nc, in_maps, core_ids=list(range(N_CORES)),
                               trace=False)
    outs = [res.results[c]["out"] for c in range(N_CORES)]
    return np.ascontiguousarray(np.concatenate(outs, axis=1))

